# revision 5
# baseline (speedup 1.0000x reference)
# Trainium2 Bass kernel for DirectSoftTreeEnsemble forward pass.
#
# Math (reference):
#   temp = clip(exp(log_temperature), 0.1, 5)
#   logits[b,t,i] = x[b,:] @ split_weights[t,i,:] + split_biases[t,i]      (i: 63 internal nodes)
#   s = sigmoid(logits / temp)
#   mu[b,t,l]     = prod over path of s / (1-s)                            (l: 64 leaves, depth 6)
#   P[t,l,:]      = softmax(leaf_logits[t,l,:] / temp)                     (C=1000 classes)
#   w             = softmax(tree_weights)                                  (T=32 trees)
#   out[b,c]      = sum_{t,l} mu[b,t,l] * w[t] * P[t,l,c]
#
# Strategy: data-parallel over batch (4096 -> 8 cores x 512 rows), tree params
# replicated.  Per core, two big matmuls on the PE array:
#   stage A: [512,1024] @ [1024,2048(ti,padded)]   (split logits)
#   stage B: [512,2048(tl)] @ [2048,1000]          (leaf blend)
# Glue on ACT (tanh/exp) and DVE (path-product doubling, softmax scale).
# sigmoid is computed via tanh so ACT needs only one function-table set:
#   2*s = 1 + tanh(z/(2*temp)),  2*(1-s) = 1 - tanh(z/(2*temp))
# The doubling then produces 64*mu; the 1/64 is folded into the leaf scale.
# The leaf softmax denominator Z comes free from the exp pass (accum_out).
#
# Host does only: sharding/layout/dtype prep, the 32-element tree softmax and
# scalar temperature value; all O(B*...)/O(T*L*C) math runs on device.

import numpy as np
import ml_dtypes

import concourse.bass as bass
import concourse.mybir as mybir
import concourse.tile as tile
from concourse import bacc
from concourse.bass_utils import run_bass_kernel_spmd

BF16 = mybir.dt.bfloat16
F32 = mybir.dt.float32
F32R = mybir.dt.float32r
AF = mybir.ActivationFunctionType
OP = mybir.AluOpType

# Problem shapes (hardcoded per contract)
B, D, C, T, DEPTH = 4096, 1024, 1000, 32, 6
NI = 2**DEPTH - 1          # 63 internal nodes / tree
L = 2**DEPTH               # 64 leaves / tree
NIP = 64                   # padded internal nodes / tree
TIP = T * NIP              # 2048 padded internal total
TL = T * L                 # 2048 leaf rows total
NCORES = 8
BS = B // NCORES           # 512 batch rows / core
MT = BS // 128             # 4 m-tiles / core
KA = D // 128              # 8 k-tiles, stage A
KB = TL // 128             # 16 k-tiles, stage B
NB_CHUNKS = [(0, 512), (512, C - 512)]  # stage-B n chunks (512, 488)


def _build(a_fp32r: bool, has_bias: bool, unit_temp: bool):
    """Build the per-core SPMD Bass program."""
    nc = bacc.Bacc("TRN2", target_bir_lowering=False, debug=False)

    a_dt = F32 if a_fp32r else BF16
    xT_d = nc.dram_tensor("xT", [D, BS], a_dt, kind="ExternalInput")
    wT_d = nc.dram_tensor("wT", [D, TIP], a_dt, kind="ExternalInput")
    ll_d = nc.dram_tensor("ll", [TL, C], BF16, kind="ExternalInput")
    wm_d = nc.dram_tensor("wm", [128, KB], F32, kind="ExternalInput")
    out_d = nc.dram_tensor("out", [BS, C], F32, kind="ExternalOutput")
    if has_bias:
        bias_d = nc.dram_tensor("biasb", [128, TIP], F32, kind="ExternalInput")
    if not unit_temp:
        lt_d = nc.dram_tensor("lt", [1, 1], F32, kind="ExternalInput")

    with tile.TileContext(nc) as tc:
        consts = tc.alloc_tile_pool(name="consts", bufs=1)
        work = tc.alloc_tile_pool(name="work", bufs=2)
        llp = tc.alloc_tile_pool(name="llp", bufs=3)
        psA = tc.alloc_tile_pool(name="psA", bufs=4, space="PSUM")
        psB = tc.alloc_tile_pool(name="psB", bufs=3, space="PSUM")

        # ---- temperature scalars -> per-partition [128,1] scale APs ----
        if unit_temp:
            ht_scale = 0.5       # tanh scale: 1/(2*temp)
            et_scale = 1.0       # exp scale: 1/temp
        else:
            ltb = consts.tile([128, 1], F32)
            nc.gpsimd.dma_start(out=ltb, in_=lt_d[:, :].partition_broadcast(128))
            tmp = consts.tile([128, 1], F32)
            nc.scalar.activation(tmp, ltb, AF.Exp)                  # temp
            nc.vector.tensor_scalar(tmp, tmp, 5.0, 0.1, OP.min, OP.max)
            itp = consts.tile([128, 1], F32)
            nc.vector.reciprocal(itp, tmp)                          # 1/temp
            htt = consts.tile([128, 1], F32)
            nc.vector.tensor_scalar_mul(htt, itp, 0.5)              # 1/(2 temp)
            ht_scale = htt[:, :]
            et_scale = itp[:, :]

        # ---- resident inputs ----
        xTs = consts.tile([128, KA, BS], a_dt)
        nc.sync.dma_start(xTs, xT_d[:, :].rearrange("(k p) b -> p k b", p=128))
        wTs = consts.tile([128, KA, TIP], a_dt)
        nc.sync.dma_start(wTs, wT_d[:, :].rearrange("(k p) n -> p k n", p=128))
        wm = consts.tile([128, KB], F32)
        nc.sync.dma_start(wm, wm_d[:, :])
        if has_bias:
            biasb = consts.tile([128, TIP], F32)
            nc.sync.dma_start(biasb, bias_d[:, :])

        # ---- leaf pipeline: P = exp(ll/temp) * w_t/(64*Z)  (bf16) ----
        P3 = consts.tile([128, KB, C], BF16)
        Z = consts.tile([128, KB], F32)
        for s in range(KB):
            llc = llp.tile([128, C], BF16, name=f"llc{s}", tag="llc")
            nc.sync.dma_start(llc, ll_d[s * 128:(s + 1) * 128, :])
            nc.scalar.activation(P3[:, s, :], llc, AF.Exp,
                                 scale=et_scale, accum_out=Z[:, s:s + 1])
        Zi = consts.tile([128, KB], F32)
        nc.vector.reciprocal(Zi, Z)
        scl = consts.tile([128, KB], F32)
        nc.vector.tensor_tensor(scl, Zi, wm, OP.mult)
        for s in range(KB):
            nc.vector.tensor_scalar_mul(P3[:, s, :], P3[:, s, :], scl[:, s:s + 1])

        # ---- per-m pipeline: stage A -> tanh -> doubling -> transpose -> stage B ----
        muT3 = consts.tile([128, KB, BS], BF16)  # mu^T, lhsT for stage B

        def mm_dt(ap):
            return ap.bitcast(F32R) if a_fp32r else ap

        for m in range(MT):
            msl = slice(m * 128, (m + 1) * 128)

            # stage A matmuls: psum[b,ti-chunk] += xT_k[:,m].T @ wT_k[:,n]
            th = work.tile([128, TIP], F32, name=f"th{m}", tag="th")
            for n in range(4):
                pa = psA.tile([128, 512], F32, name=f"pa{m}_{n}", tag="pa")
                for k in range(KA):
                    nc.tensor.matmul(
                        pa, mm_dt(xTs[:, k, msl]),
                        mm_dt(wTs[:, k, n * 512:(n + 1) * 512]),
                        start=(k == 0), stop=(k == KA - 1))
                nsl = slice(n * 512, (n + 1) * 512)
                if has_bias:
                    nc.vector.tensor_tensor(pa, pa, biasb[:, nsl], OP.add)
                nc.scalar.activation(th[:, nsl], pa, AF.Tanh, scale=ht_scale)

            omt = work.tile([128, TIP], F32, name=f"omt{m}", tag="omt")
            nc.vector.tensor_scalar(omt, th, -1.0, 1.0, OP.mult, OP.add)

            # path-product doubling in BFS order (factors 1±tanh = 2s / 2(1-s))
            th3 = th.rearrange("p (t i) -> p t i", t=T)
            om3 = omt.rearrange("p (t i) -> p t i", t=T)
            muA = work.tile([128, T * 32], F32, name=f"muA{m}", tag="muA")
            muB = work.tile([128, T * 32], F32, name=f"muB{m}", tag="muB")
            mu6 = work.tile([128, TL], BF16, name=f"mu6{m}", tag="mu6")

            def lvl_view(d):
                # mu_d laid out [p, t, 2^d]; odd levels in muA, even in muB
                buf = muA if d % 2 == 1 else muB
                return buf[:, :T * (2 ** d)].rearrange("p (t j) -> p t j", t=T)

            # level 0: mu1[t, 0] = 1-th[t,0] ; mu1[t, 1] = 1+th[t,0]
            mu1 = lvl_view(1).rearrange("p t (j two) -> p t j two", two=2)
            nc.vector.tensor_scalar_add(mu1[:, :, 0, 0], om3[:, :, 0], 0.0)
            nc.vector.tensor_scalar_add(mu1[:, :, 0, 1], th3[:, :, 0], 1.0)

            for d in range(1, DEPTH):
                lo, hi = 2 ** d - 1, 2 ** (d + 1) - 1
                mu_d = lvl_view(d)
                if d == DEPTH - 1:
                    dst = mu6.rearrange("p (t j) -> p t j", t=T)
                else:
                    dst = lvl_view(d + 1)
                d2 = dst.rearrange("p t (j two) -> p t j two", two=2)
                # left child: mu * (1 - th)
                nc.vector.tensor_tensor(
                    d2[:, :, :, 0], mu_d, om3[:, :, lo:hi], OP.mult)
                # right child: (th + 1) * mu
                nc.vector.scalar_tensor_tensor(
                    d2[:, :, :, 1], th3[:, :, lo:hi], 1.0, mu_d,
                    OP.add, OP.mult)

            # transpose mu (bf16) into muT via DMA xbar
            for k in range(KB):
                nc.sync.dma_start_transpose(
                    muT3[:, k, msl], mu6[:, k * 128:(k + 1) * 128])

            # stage B matmuls + evacuate + store
            outm = work.tile([128, C], F32, name=f"outm{m}", tag="outm")
            for (c0, cn) in NB_CHUNKS:
                pb = psB.tile([128, 512], F32, name=f"pb{m}_{c0}", tag="pb")
                for k in range(KB):
                    nc.tensor.matmul(
                        pb[:, :cn], muT3[:, k, msl], P3[:, k, c0:c0 + cn],
                        start=(k == 0), stop=(k == KB - 1))
                nc.any.tensor_copy(outm[:, c0:c0 + cn], pb[:, :cn])
            nc.sync.dma_start(out_d[msl, :], outm)

        psB.release()
        psA.release()
        llp.release()
        work.release()
        consts.release()

    nc.compile()
    return nc


_cache = {}


def _get_nc(key):
    if key not in _cache:
        _cache[key] = _build(*key)
    return _cache[key]


A_FP32R = False  # stage-A matmul dtype lever (False = bf16)


def kernel(x, split_weights, split_biases, leaf_logits, tree_weights,
           log_temperature):
    x = np.asarray(x, np.float32)
    split_weights = np.asarray(split_weights, np.float32)
    split_biases = np.asarray(split_biases, np.float32)
    leaf_logits = np.asarray(leaf_logits, np.float32)
    tree_weights = np.asarray(tree_weights, np.float32)
    lt = float(np.asarray(log_temperature, np.float32).reshape(-1)[0])

    has_bias = bool(np.any(split_biases != 0.0))
    unit_temp = (lt == 0.0)
    a_fp32r = A_FP32R
    a_np = np.float32 if a_fp32r else ml_dtypes.bfloat16

    # ---- host layout prep ----
    # W^T [D, TIP]: pad 63->64 nodes per tree with zero rows
    wpad = np.zeros((T, NIP, D), np.float32)
    wpad[:, :NI, :] = split_weights
    wT = np.ascontiguousarray(wpad.reshape(TIP, D).T.astype(a_np))
    # x^T shards [D, BS] per core
    xT = x.T.astype(a_np)
    xT_shards = [np.ascontiguousarray(xT[:, c * BS:(c + 1) * BS])
                 for c in range(NCORES)]
    # leaf logits [TL, C] bf16 (BFS leaf order; no permutation needed)
    ll = np.ascontiguousarray(leaf_logits.reshape(TL, C).astype(ml_dtypes.bfloat16))
    # tree-weight softmax (32 scalars on host) folded with the 1/64 doubling fixup
    twf = tree_weights - tree_weights.max()
    w = np.exp(twf) / np.exp(twf).sum()
    w64 = (w / 64.0).astype(np.float32)
    # wm[p, s] = w[(s*128+p)//64] / 64
    p_idx = np.arange(128)[:, None]
    s_idx = np.arange(KB)[None, :]
    wm = np.ascontiguousarray(w64[(s_idx * 128 + p_idx) // 64])

    in_map_common = {"wT": wT, "ll": ll, "wm": wm}
    if has_bias:
        bpad = np.zeros((T, NIP), np.float32)
        bpad[:, :NI] = split_biases
        in_map_common["biasb"] = np.ascontiguousarray(
            np.broadcast_to(bpad.reshape(1, TIP), (128, TIP)).astype(np.float32))
    if not unit_temp:
        in_map_common["lt"] = np.full((1, 1), lt, np.float32)

    nc = _get_nc((a_fp32r, has_bias, unit_temp))
    in_maps = [{"xT": xT_shards[c], **in_map_common} for c in range(NCORES)]
    res = run_bass_kernel_spmd(nc, in_maps, core_ids=list(range(NCORES)))
    global LAST_RESULT
    LAST_RESULT = res
    out = np.concatenate([r["out"] for r in res.results], axis=0)
    return np.ascontiguousarray(out.astype(np.float32))


LAST_RESULT = None
